# revision 26
# baseline (speedup 1.0000x reference)
"""Trainium2 Bass kernel for a dense transformer block (self-contained).

Block: x + attn(x) -> rmsnorm -> + swiglu-mlp -> rmsnorm
Shapes: B=2, S=2048, D=2048, H=16 (hd=128), HIDDEN=5632, fp32 I/O.

Sharding over 8 NeuronCores:
  - x enters token-sharded ([512, 2048] f32 per core, a contiguous row
    slice of the flattened [4096, 2048] x — zero host-side prep).
  - QKV is computed token-parallel (each core: its 512 tokens, all 16
    heads), RoPE applied, then one AllToAll reshards q,k,v to
    head-parallel (each core: 2 heads, all 4096 tokens).
  - Attention (block-causal softmax) runs head-parallel; a second
    AllToAll reshards the context back to token-parallel.
  - wo projection, rmsnorms and the MLP are token-parallel with full
    (replicated, device-resident) weights streamed from DRAM.
  - All post-attention activations are feature-major [feat_part,
    tok_free]; rmsnorm partition reductions use ones-matmuls on the PE.
  - x is transposed to feature-major on device (PE transposes); the
    final result is transposed back so the output is token-major
    [512, 2048] f32 per core => concat over cores == [4096, 2048].
  - Weights are pre-swizzled on host ONCE (cached) into layouts whose
    per-partition DMA lines are 4KB+ contiguous, then kept device-
    resident across calls. Per-call H2D is x only (32 MB).
  - All matmul operands are bf16 (PE streams 1 elem/cycle).
"""
import numpy as np

import concourse.bacc as bacc
import concourse.bass as bass
import concourse.tile as tile
import concourse.mybir as mybir

F32 = mybir.dt.float32
BF16 = mybir.dt.bfloat16
AF = mybir.ActivationFunctionType

NCORES = 8
B, S, D = 2, 2048, 2048
H, HD = 16, 128
HID = 5632
NT = B * S              # 4096 tokens global
TPC = NT // NCORES      # 512 tokens per core
HPC = H // NCORES       # 2 heads per core
KD = D // 128           # 16 feature chunks
KH = HID // 128         # 44 hidden chunks
NJ = NT // 512          # 8 global token chunks of 512
QC = S // 512           # 4 q-chunks per batch
TT = TPC // 128         # 4 local token tiles
EPS = 1e-6
ISQ = 1.0 / np.sqrt(HD)

# cons (bf16) column layout
C_RMAT = 0              # [0,128)    rope half-swap matrix
C_MASK = 128            # [128,2176) 4 x [128,512] causal masks
C_ONES = 2176           # [2176,2304) ones block
NCONS = 2304
# consf (f32) column layout
CF_ID = 0               # [0,128)    identity (PE transpose)
CF_N1 = 128             # [128,144)  norm1 weight [128, KD]
CF_N2 = 144             # [144,160)  norm2 weight
NCONSF = 160

_RT: dict = {}
PHASE_LIMIT = None   # dev hook: build only phases <= this (None = all)


def _ph(n):
    return PHASE_LIMIT is None or n <= PHASE_LIMIT


# --------------------------------------------------------------------------
# device program
# --------------------------------------------------------------------------

def _build_nc(reps=1):
    nc = bacc.Bacc("TRN2", target_bir_lowering=False, debug=False,
                   num_devices=NCORES)

    xin = nc.dram_tensor("xin", [TPC, D], F32, kind="ExternalInput")
    wqk = nc.dram_tensor("wqk", [128, 2, H, KD, 128], BF16,
                         kind="ExternalInput")
    wv = nc.dram_tensor("wv", [128, KD, D], BF16, kind="ExternalInput")
    wo = nc.dram_tensor("wo", [128, KD, KD, 128], BF16, kind="ExternalInput")
    w1v1 = nc.dram_tensor("w1v1", [128, 2, KH, KD, 128], BF16,
                          kind="ExternalInput")
    w2 = nc.dram_tensor("w2", [128, KD, KH, 128], BF16, kind="ExternalInput")
    atab = nc.dram_tensor("atab", [HD, 512], BF16, kind="ExternalInput")
    btab = nc.dram_tensor("btab", [HD, 512], BF16, kind="ExternalInput")
    cons = nc.dram_tensor("cons", [128, NCONS], BF16, kind="ExternalInput")
    consf = nc.dram_tensor("consf", [128, NCONSF], F32, kind="ExternalInput")

    out = nc.dram_tensor("out", [TPC, D], F32, kind="ExternalOutput")

    with tile.TileContext(nc) as tc:
        # ---- persistent constants (loaded once; ~1MB) -----------------
        const = tc.alloc_tile_pool(name="const", bufs=1)
        cons_sb = const.tile([128, NCONS], BF16, tag="cons")
        nc.sync.dma_start(cons_sb[:], cons[:])
        consf_sb = const.tile([128, NCONSF], F32, tag="consf")
        nc.sync.dma_start(consf_sb[:], consf[:])
        atab_sb = const.tile([HD, 512], BF16, tag="atab")
        nc.sync.dma_start(atab_sb[:], atab[:])
        btab_sb = const.tile([HD, 512], BF16, tag="btab")
        nc.sync.dma_start(btab_sb[:], btab[:])
        epsc = const.tile([1, 1], F32, tag="epsc")
        nc.vector.memset(epsc[:], EPS)

        rmat_sb = cons_sb[:, C_RMAT:C_RMAT + 128]
        ident_sb = consf_sb[:, CF_ID:CF_ID + 128]
        onesk = cons_sb[:, C_ONES:C_ONES + 1]            # [128, 1]
        onesm = cons_sb[0:1, C_ONES:C_ONES + 128]        # [1, 128]

        def mask_sb(m):
            return cons_sb[:, C_MASK + 512 * m:C_MASK + 512 * (m + 1)]

        for rep in range(reps):
            a1qi = nc.dram_tensor(f"a1qi{rep}", [NCORES, 4, HD, 512], BF16)
            a1qo = nc.dram_tensor(f"a1qo{rep}", [NCORES, 4, HD, 512], BF16)
            a1vi = nc.dram_tensor(f"a1vi{rep}", [NCORES, 2, HD, 512], BF16)
            a1vo = nc.dram_tensor(f"a1vo{rep}", [NCORES, 2, HD, 512], BF16)
            a2i = nc.dram_tensor(f"a2i{rep}", [NCORES, HPC, HD, 512], BF16)
            a2o = nc.dram_tensor(f"a2o{rep}", [NCORES, HPC, HD, 512], BF16)

            # yt lives P3->P5; allocate below xpool so releases stay LIFO
            post = tc.alloc_tile_pool(name=f"post{rep}", bufs=1)
            yt = [post.tile([128, 512], BF16, tag=f"y{m}", name=f"y{m}")
                  for m in range(KD)]

            # ============ Phase 0: load x, transpose on PE =============
            xpool = tc.alloc_tile_pool(name=f"xp{rep}", bufs=1)
            xT32 = [xpool.tile([128, TPC], F32, tag=f"x32_{k}",
                               name=f"x32_{k}") for k in range(KD)]
            xbfp = tc.alloc_tile_pool(name=f"xbf{rep}", bufs=1)
            xTbf = [xbfp.tile([128, TPC], BF16, tag=f"xbf_{k}",
                              name=f"xbf_{k}") for k in range(KD)]
            with (
                tc.tile_pool(name=f"xn{rep}", bufs=2) as xn_pool,
                tc.tile_pool(name=f"pst0{rep}", bufs=4, space="PSUM") as pst0,
            ):
                for t in range(TT):
                    xn = xn_pool.tile([128, D], F32, tag="xn")
                    nc.sync.dma_start(xn[:], xin[128 * t:128 * (t + 1), :])
                    for k in range(KD):
                        ps = pst0.tile([128, 128], F32, tag="ps")
                        nc.tensor.transpose(ps[:], xn[:, 128 * k:128 * (k + 1)],
                                            ident_sb)
                        nc.scalar.activation(
                            xT32[k][:, 128 * t:128 * (t + 1)], ps[:], AF.Copy)
                for k in range(KD):
                    nc.vector.tensor_copy(xTbf[k][:], xT32[k][:])

            # ============ Phase 1: QKV (token-parallel) + RoPE =========
            with (
                tc.tile_pool(name=f"wvp{rep}", bufs=1) as wv_pool,
                tc.tile_pool(name=f"wqkp{rep}", bufs=4) as wqk_pool,
                tc.tile_pool(name=f"scr1{rep}", bufs=4) as scr1,
                tc.tile_pool(name=f"pqk{rep}", bufs=2, space="PSUM") as pqk,
                tc.tile_pool(name=f"psr{rep}", bufs=2, space="PSUM") as psrp,
                tc.tile_pool(name=f"psv{rep}", bufs=2, space="PSUM") as psvp,
            ):
                wv_sb = wv_pool.tile([128, KD * D], BF16, tag="wv")
                nc.gpsimd.dma_start(
                    wv_sb[:].rearrange("p (k n) -> p k n", k=KD), wv[:])

                for h in range(H):
                    wq_t = wqk_pool.tile([128, KD * 128], BF16, tag="wq")
                    nc.gpsimd.dma_start(
                        wq_t[:].rearrange("p (k n) -> p k n", k=KD),
                        wqk[:, 0, h])
                    wk_t = wqk_pool.tile([128, KD * 128], BF16, tag="wk")
                    nc.gpsimd.dma_start(
                        wk_t[:].rearrange("p (k n) -> p k n", k=KD),
                        wqk[:, 1, h])
                    psq = pqk.tile([128, 512], F32, tag="psq")
                    psk = pqk.tile([128, 512], F32, tag="psk")
                    for k in range(KD):
                        nc.tensor.matmul(psq[:],
                                         wq_t[:, 128 * k:128 * (k + 1)],
                                         xTbf[k][:], start=(k == 0),
                                         stop=(k == KD - 1))
                    for k in range(KD):
                        nc.tensor.matmul(psk[:],
                                         wk_t[:, 128 * k:128 * (k + 1)],
                                         xTbf[k][:], start=(k == 0),
                                         stop=(k == KD - 1))
                    # rope q into ro[:, 0:512], k into ro[:, 512:1024];
                    # one staging DMA covers slots 2*hl (q) and 2*hl+1 (k)
                    ro = scr1.tile([128, 1024], BF16, tag="ro")
                    for ps, half in ((psq, 0), (psk, 1)):
                        raw = scr1.tile([128, 512], BF16, tag="raw")
                        nc.scalar.activation(raw[:], ps[:], AF.Copy)
                        psr = psrp.tile([128, 512], F32, tag="psr")
                        nc.tensor.matmul(psr[:], rmat_sb, raw[:],
                                         start=True, stop=True)
                        t1 = scr1.tile([128, 512], BF16, tag="t1")
                        nc.vector.tensor_mul(t1[:], raw[:], atab_sb[:])
                        t2 = scr1.tile([128, 512], BF16, tag="t2")
                        nc.vector.tensor_mul(t2[:], btab_sb[:], psr[:])
                        nc.vector.tensor_add(
                            ro[:, 512 * half:512 * (half + 1)], t1[:], t2[:])
                    nc.sync.dma_start(
                        a1qi[h // 2, 2 * (h % 2):2 * (h % 2) + 2]
                        .rearrange("s p n -> p s n"), ro[:])

                # q/k exchange fires now; v compute + exchange overlap it
                nc.gpsimd.collective_compute(
                    "AllToAll", mybir.AluOpType.bypass,
                    replica_groups=[list(range(NCORES))],
                    ins=[a1qi[:].opt()], outs=[a1qo[:].opt()])

                # v natural (token-major), 4 heads (512 feats) at a time
                for fg in range(4):
                    for t in range(TT):
                        psv = psvp.tile([128, 512], F32, tag="psv")
                        for k in range(KD):
                            nc.tensor.matmul(
                                psv[:], xTbf[k][:, 128 * t:128 * (t + 1)],
                                wv_sb[:, k * D + 512 * fg:k * D + 512 * (fg + 1)],
                                start=(k == 0), stop=(k == KD - 1))
                        vsb = scr1.tile([128, 512], BF16, tag="vsb")
                        nc.scalar.activation(vsb[:], psv[:], AF.Copy)
                        nc.sync.dma_start(
                            a1vi[2 * fg:2 * fg + 2, t // 2, :,
                                 256 * (t % 2):256 * (t % 2) + 256]
                            .rearrange("j p n -> p j n"), vsb[:])

            xbfp.release()

            # ============ AllToAll #1b: v -> head-parallel =============
            nc.gpsimd.collective_compute(
                "AllToAll", mybir.AluOpType.bypass,
                replica_groups=[list(range(NCORES))],
                ins=[a1vi[:].opt()], outs=[a1vo[:].opt()])

            # ============ Phase 2: attention (head-parallel) ===========
            attn = tc.alloc_tile_pool(name=f"at{rep}", bufs=1)
            # qkT columns: ((s*NJ + j)*512 + n), s = 2*hl + (0=q, 1=k)
            qkT = attn.tile([128, 4 * NJ * 512], BF16, tag="qkT")
            vN = attn.tile([128, (NT // 128) * HPC * HD], BF16, tag="vN")
            for s in range(4):
                nc.gpsimd.dma_start(
                    qkT[:, s * NJ * 512:(s + 1) * NJ * 512]
                    .rearrange("p (j n) -> p j n", j=NJ),
                    a1qo[:, s].rearrange("j p n -> p j n"))
            for c in range(2):
                nc.gpsimd.dma_start(
                    vN[:].rearrange("p (j c n) -> p c j n", j=NJ, c=2)[:, c],
                    a1vo[:, c].rearrange("j p n -> p j n"))
            qT = [[qkT[:, ((2 * hl) * NJ + j) * 512:
                        ((2 * hl) * NJ + j) * 512 + 512] for j in range(NJ)]
                  for hl in range(HPC)]
            kT = [[qkT[:, ((2 * hl + 1) * NJ + j) * 512:
                        ((2 * hl + 1) * NJ + j) * 512 + 512] for j in range(NJ)]
                  for hl in range(HPC)]

            with (
                tc.tile_pool(name=f"pr{rep}", bufs=8) as probs_pool,
                tc.tile_pool(name=f"bscr{rep}", bufs=3) as bscr,
                tc.tile_pool(name=f"pss{rep}", bufs=3, space="PSUM") as pss_p,
                tc.tile_pool(name=f"psd{rep}", bufs=2, space="PSUM") as psd_p,
                tc.tile_pool(name=f"pso{rep}", bufs=2, space="PSUM") as pso_p,
                tc.tile_pool(name=f"psb{rep}", bufs=1, space="PSUM") as psb_p,
            ):
                for b in range(B):
                    for hl in range(HPC):
                        for qc in range(QC):
                            j = QC * b + qc
                            nkt = 4 * (qc + 1)
                            psd = psd_p.tile([1, 512], F32, tag="psd")
                            pso = pso_p.tile([128, 512], F32, tag="pso")
                            for kt in range(nkt):
                                jk = QC * b + kt // 4
                                ksl = kT[hl][jk][:, 128 * (kt % 4):
                                                 128 * (kt % 4 + 1)]
                                pss = pss_p.tile([128, 512], F32, tag="pss")
                                nc.tensor.matmul(pss[:], ksl, qT[hl][j][:],
                                                 start=True, stop=True)
                                prob = probs_pool.tile([128, 512], BF16,
                                                       tag="pr")
                                m = kt - 4 * qc
                                if m >= 0:
                                    esc = bscr.tile([128, 512], BF16, tag="e")
                                    nc.scalar.activation(esc[:], pss[:],
                                                         AF.Exp, scale=ISQ)
                                    nc.vector.tensor_mul(prob[:], esc[:],
                                                         mask_sb(m))
                                else:
                                    nc.scalar.activation(prob[:], pss[:],
                                                         AF.Exp, scale=ISQ)
                                nc.tensor.matmul(psd[:], onesk, prob[:],
                                                 start=(kt == 0),
                                                 stop=(kt == nkt - 1))
                                g = 16 * b + kt
                                vsl = vN[:, 256 * g + 128 * hl:
                                         256 * g + 128 * (hl + 1)]
                                nc.tensor.matmul(pso[:], vsl, prob[:],
                                                 start=(kt == 0),
                                                 stop=(kt == nkt - 1))
                            rd = bscr.tile([1, 512], BF16, tag="rd")
                            with nc.allow_low_precision(reason="softmax recip"):
                                nc.vector.reciprocal(rd[:], psd[:])
                            psb = psb_p.tile([128, 512], F32, tag="psb")
                            nc.tensor.matmul(psb[:], onesm, rd[:],
                                             start=True, stop=True)
                            rb = bscr.tile([128, 512], F32, tag="rb")
                            nc.vector.tensor_copy(rb[:], psb[:])
                            osb = bscr.tile([128, 512], BF16, tag="osb")
                            nc.vector.tensor_mul(osb[:], rb[:], pso[:])
                            nc.sync.dma_start(a2i[j, hl], osb[:])

            attn.release()

            # ============ AllToAll #2: context -> token-parallel =======
            nc.gpsimd.collective_compute(
                "AllToAll", mybir.AluOpType.bypass,
                replica_groups=[list(range(NCORES))],
                ins=[a2i[:].opt()], outs=[a2o[:].opt()])

            # ============ Phase 3: wo + residual + rmsnorm1 ============
            with (
                tc.tile_pool(name=f"oT{rep}", bufs=1) as oT_pool,
                tc.tile_pool(name=f"wop{rep}", bufs=2) as wo_pool,
                tc.tile_pool(name=f"ht{rep}", bufs=1) as ht_pool,
                tc.tile_pool(name=f"dscr{rep}", bufs=3) as dscr,
                tc.tile_pool(name=f"psh{rep}", bufs=2, space="PSUM") as psh_p,
                tc.tile_pool(name=f"psn{rep}", bufs=2, space="PSUM") as psn_p,
            ):
                oTb = oT_pool.tile([128, KD * 512], BF16, tag="oTb")
                for hl in range(HPC):
                    nc.gpsimd.dma_start(
                        oTb[:].rearrange("p (j h n) -> p h j n",
                                         j=NCORES, h=HPC)[:, hl],
                        a2o[:, hl].rearrange("j p n -> p j n"))
                oT = [oTb[:, 512 * r:512 * (r + 1)] for r in range(KD)]

                ht = []
                psss = psn_p.tile([1, 512], F32, tag="ss")
                for m in range(KD):
                    wos = wo_pool.tile([128, KD * 128], BF16, tag="wos")
                    nc.gpsimd.dma_start(
                        wos[:].rearrange("p (r n) -> p r n", r=KD), wo[:, m])
                    psh = psh_p.tile([128, 512], F32, tag="psh")
                    for r in range(KD):
                        nc.tensor.matmul(psh[:], wos[:, 128 * r:128 * (r + 1)],
                                         oT[r][:], start=(r == 0),
                                         stop=(r == KD - 1))
                    h_sb = ht_pool.tile([128, 512], F32, tag=f"h{m}",
                                        name=f"h{m}")
                    nc.vector.tensor_add(h_sb[:], xT32[m][:], psh[:])
                    ht.append(h_sb)
                    sq = dscr.tile([128, 512], BF16, tag="sq")
                    nc.vector.tensor_mul(sq[:], h_sb[:], h_sb[:])
                    nc.tensor.matmul(psss[:], onesk, sq[:],
                                     start=(m == 0), stop=(m == KD - 1))

                u = dscr.tile([1, 512], F32, tag="u")
                nc.scalar.activation(u[:], psss[:], AF.Sqrt, scale=1.0 / D,
                                     bias=epsc[:])
                rs = dscr.tile([1, 512], BF16, tag="rs")
                with nc.allow_low_precision(reason="rmsnorm recip"):
                    nc.vector.reciprocal(rs[:], u[:])
                psb1 = psn_p.tile([128, 512], F32, tag="bc")
                nc.tensor.matmul(psb1[:], onesm, rs[:], start=True, stop=True)
                rb1 = dscr.tile([128, 512], F32, tag="rb1")
                nc.scalar.activation(rb1[:], psb1[:], AF.Copy)
                for m in range(KD):
                    ytmp = dscr.tile([128, 512], F32, tag="ytmp")
                    nc.vector.tensor_mul(ytmp[:], ht[m][:], rb1[:])
                    nc.scalar.activation(yt[m][:], ytmp[:], AF.Copy,
                                         scale=consf_sb[:, CF_N1 + m:
                                                        CF_N1 + m + 1])

            xpool.release()

            # ============ Phase 4: MLP up (token-parallel) =============
            mlp = tc.alloc_tile_pool(name=f"mlp{rep}", bufs=1)
            mt = [mlp.tile([128, 512], BF16, tag=f"m{t}", name=f"m{t}")
                  for t in range(KH)]
            with (
                tc.tile_pool(name=f"w1p{rep}", bufs=3) as w1_pool,
                tc.tile_pool(name=f"v1p{rep}", bufs=3) as v1_pool,
                tc.tile_pool(name=f"escr{rep}", bufs=3) as escr,
                tc.tile_pool(name=f"ps1{rep}", bufs=2, space="PSUM") as ps1_p,
                tc.tile_pool(name=f"ps2{rep}", bufs=2, space="PSUM") as ps2_p,
            ):
                for t in range(KH):
                    w1s = w1_pool.tile([128, KD * 128], BF16, tag="w1s")
                    nc.gpsimd.dma_start(
                        w1s[:].rearrange("p (k n) -> p k n", k=KD),
                        w1v1[:, 0, t])
                    v1s = v1_pool.tile([128, KD * 128], BF16, tag="v1s")
                    nc.gpsimd.dma_start(
                        v1s[:].rearrange("p (k n) -> p k n", k=KD),
                        w1v1[:, 1, t])
                    ps1 = ps1_p.tile([128, 512], F32, tag="ps1")
                    ps2 = ps2_p.tile([128, 512], F32, tag="ps2")
                    for k in range(KD):
                        nc.tensor.matmul(ps1[:], w1s[:, 128 * k:128 * (k + 1)],
                                         yt[k][:], start=(k == 0),
                                         stop=(k == KD - 1))
                    for k in range(KD):
                        nc.tensor.matmul(ps2[:], v1s[:, 128 * k:128 * (k + 1)],
                                         yt[k][:], start=(k == 0),
                                         stop=(k == KD - 1))
                    ssc = escr.tile([128, 512], BF16, tag="ssc")
                    nc.scalar.activation(ssc[:], ps1[:], AF.Silu)
                    nc.vector.tensor_mul(mt[t][:], ssc[:], ps2[:])

            # ====== Phase 5: MLP down + residual + rmsnorm2 + out ======
            with (
                tc.tile_pool(name=f"w2p{rep}", bufs=3) as w2_pool,
                tc.tile_pool(name=f"o2p{rep}", bufs=1) as o2_pool,
                tc.tile_pool(name=f"on{rep}", bufs=1) as on_pool,
                tc.tile_pool(name=f"fscr{rep}", bufs=3) as fscr,
                tc.tile_pool(name=f"pso2{rep}", bufs=2, space="PSUM") as pso2_p,
                tc.tile_pool(name=f"psn2{rep}", bufs=2, space="PSUM") as psn2_p,
                tc.tile_pool(name=f"pst6{rep}", bufs=2, space="PSUM") as pst6,
            ):
                psss2 = psn2_p.tile([1, 512], F32, tag="ss2")
                o2l = []
                for m in range(KD):
                    w2s = w2_pool.tile([128, KH * 128], BF16, tag="w2s")
                    nc.gpsimd.dma_start(
                        w2s[:].rearrange("p (t n) -> p t n", t=KH), w2[:, m])
                    pso2 = pso2_p.tile([128, 512], F32, tag="pso2")
                    for t in range(KH):
                        nc.tensor.matmul(pso2[:],
                                         w2s[:, 128 * t:128 * (t + 1)],
                                         mt[t][:], start=(t == 0),
                                         stop=(t == KH - 1))
                    o2 = o2_pool.tile([128, 512], F32, tag=f"o2{m}",
                                      name=f"o2{m}")
                    nc.vector.tensor_add(o2[:], yt[m][:], pso2[:])
                    o2l.append(o2)
                    sq2 = fscr.tile([128, 512], BF16, tag="sq2")
                    nc.vector.tensor_mul(sq2[:], o2[:], o2[:])
                    nc.tensor.matmul(psss2[:], onesk, sq2[:],
                                     start=(m == 0), stop=(m == KD - 1))

                u2 = fscr.tile([1, 512], F32, tag="u2")
                nc.scalar.activation(u2[:], psss2[:], AF.Sqrt, scale=1.0 / D,
                                     bias=epsc[:])
                rs2 = fscr.tile([1, 512], BF16, tag="rs2")
                with nc.allow_low_precision(reason="rmsnorm recip"):
                    nc.vector.reciprocal(rs2[:], u2[:])
                psb2 = psn2_p.tile([128, 512], F32, tag="bc2")
                nc.tensor.matmul(psb2[:], onesm, rs2[:], start=True, stop=True)
                rb2 = fscr.tile([128, 512], F32, tag="rb2")
                nc.scalar.activation(rb2[:], psb2[:], AF.Copy)

                outn = [on_pool.tile([128, D], F32, tag=f"on{t}",
                                     name=f"on{t}") for t in range(TT)]
                for m in range(KD):
                    fo = fscr.tile([128, 512], F32, tag="fo")
                    nc.vector.tensor_mul(fo[:], o2l[m][:], rb2[:])
                    fo2 = fscr.tile([128, 512], F32, tag="fo2")
                    nc.scalar.activation(fo2[:], fo[:], AF.Copy,
                                         scale=consf_sb[:, CF_N2 + m:
                                                        CF_N2 + m + 1])
                    for t in range(TT):
                        ps = pst6.tile([128, 128], F32, tag="ps6")
                        nc.tensor.transpose(ps[:],
                                            fo2[:, 128 * t:128 * (t + 1)],
                                            ident_sb)
                        nc.scalar.activation(
                            outn[t][:, 128 * m:128 * (m + 1)], ps[:], AF.Copy)
                for t in range(TT):
                    nc.sync.dma_start(out[128 * t:128 * (t + 1), :], outn[t][:])

            mlp.release()
            post.release()
        const.release()

    nc.compile()
    return nc


# --------------------------------------------------------------------------
# host side: weight prep (cached), runner, kernel()
# --------------------------------------------------------------------------

WNAMES = ("wq", "wk", "wv", "wo", "w_mlp", "v_mlp", "w2_mlp",
          "norm1_w", "norm2_w", "freqs_cos", "freqs_sin")


def _prep_weight_maps(inp):
    """Host-side weight swizzle -> concatenated (8*rows) arrays, one per
    device input tensor (minus xin). Runs once per distinct weight set."""
    import ml_dtypes
    bf = ml_dtypes.bfloat16
    f32 = np.float32

    wq = np.asarray(inp["wq"], f32)
    wk = np.asarray(inp["wk"], f32)
    wvv = np.asarray(inp["wv"], f32)
    wo = np.asarray(inp["wo"], f32)
    w_mlp = np.asarray(inp["w_mlp"], f32)
    v_mlp = np.asarray(inp["v_mlp"], f32)
    w2_mlp = np.asarray(inp["w2_mlp"], f32)

    perm = np.concatenate([np.arange(0, HD, 2), np.arange(1, HD, 2)])
    wqp = wq.reshape(D, H, HD)[:, :, perm]
    wkp = wk.reshape(D, H, HD)[:, :, perm]
    wqk_arr = np.empty((128, 2, H, KD, 128), bf)
    wqk_arr[:, 0] = wqp.reshape(KD, 128, H, 128).transpose(1, 2, 0, 3)
    wqk_arr[:, 1] = wkp.reshape(KD, 128, H, 128).transpose(1, 2, 0, 3)

    wv_arr = np.ascontiguousarray(
        wvv.reshape(KD, 128, D).transpose(1, 0, 2)).astype(bf)
    wo_arr = np.ascontiguousarray(
        wo.reshape(KD, 128, KD, 128).transpose(1, 2, 0, 3)).astype(bf)
    w1v1_arr = np.empty((128, 2, KH, KD, 128), bf)
    w1v1_arr[:, 0] = w_mlp.reshape(KH, 128, KD, 128).transpose(3, 0, 2, 1)
    w1v1_arr[:, 1] = v_mlp.reshape(KH, 128, KD, 128).transpose(3, 0, 2, 1)
    w2_arr = np.ascontiguousarray(
        w2_mlp.reshape(KD, 128, KH, 128).transpose(3, 0, 2, 1)).astype(bf)

    cosT = np.asarray(inp["freqs_cos"], f32).T          # [64, S]
    sinT = np.asarray(inp["freqs_sin"], f32).T
    atab_full = np.concatenate([cosT, cosT], axis=0).astype(bf)   # [128, S]
    btab_full = np.concatenate([-sinT, sinT], axis=0).astype(bf)

    rmat = np.zeros((HD, HD), f32)
    e = np.arange(64)
    rmat[e, 64 + e] = 1.0
    rmat[64 + e, e] = 1.0
    m_idx = np.arange(4)[:, None, None]
    p_idx = np.arange(128)[None, :, None]
    f_idx = np.arange(512)[None, None, :]
    masks = (128 * m_idx + p_idx <= f_idx).astype(f32)   # [4,128,512]
    cons = np.zeros((128, NCONS), f32)
    cons[:, C_RMAT:C_RMAT + 128] = rmat
    cons[:, C_MASK:C_MASK + 2048] = masks.transpose(1, 0, 2).reshape(128, 2048)
    cons[:, C_ONES:C_ONES + 128] = 1.0
    cons = cons.astype(bf)

    consf = np.zeros((128, NCONSF), f32)
    consf[:, CF_ID:CF_ID + 128] = np.eye(128, dtype=f32)
    consf[:, CF_N1:CF_N1 + KD] = np.asarray(
        inp["norm1_w"], f32).reshape(KD, 128).T
    consf[:, CF_N2:CF_N2 + KD] = np.asarray(
        inp["norm2_w"], f32).reshape(KD, 128).T

    def rep(a):  # replicate across cores along axis 0
        return np.concatenate([a] * NCORES, axis=0)

    atab_cat = np.concatenate(
        [atab_full[:, 512 * (i % QC):512 * (i % QC + 1)] for i in range(NCORES)],
        axis=0)
    btab_cat = np.concatenate(
        [btab_full[:, 512 * (i % QC):512 * (i % QC + 1)] for i in range(NCORES)],
        axis=0)

    return {
        "wqk": rep(wqk_arr), "wv": rep(wv_arr), "wo": rep(wo_arr),
        "w1v1": rep(w1v1_arr), "w2": rep(w2_arr),
        "atab": atab_cat, "btab": btab_cat,
        "cons": rep(cons), "consf": rep(consf),
    }


def _fingerprint(inp):
    parts = []
    for k in WNAMES:
        a = np.asarray(inp[k])
        flat = a.reshape(-1)
        step = max(1, flat.shape[0] // 1024)
        parts.append((k, a.shape, str(a.dtype),
                      flat[::step][:1024].tobytes(),
                      flat[:64].tobytes(), flat[-64:].tobytes()))
    import hashlib
    hsh = hashlib.sha1()
    for p in parts:
        hsh.update(repr(p[:3]).encode())
        hsh.update(p[3]); hsh.update(p[4]); hsh.update(p[5])
    return hsh.hexdigest()


def _make_runner(nc):
    """Build a non-donating sharded jit for a compiled nc. Returns
    (sharded, in_names, out_avals, mesh, sh)."""
    import jax
    from jax.sharding import Mesh, PartitionSpec, NamedSharding
    from jax.experimental.shard_map import shard_map
    from concourse.bass2jax import (_bass_exec_p, install_neuronx_cc_hook,
                                    partition_id_tensor)
    install_neuronx_cc_hook()

    partition_name = (nc.partition_id_tensor.name
                      if nc.partition_id_tensor else None)
    in_names, out_names, out_avals = [], [], []
    for alloc in nc.m.functions[0].allocations:
        if not isinstance(alloc, mybir.MemoryLocationSet):
            continue
        name = alloc.memorylocations[0].name
        if alloc.kind == "ExternalInput":
            if name != partition_name:
                in_names.append(name)
        elif alloc.kind == "ExternalOutput":
            out_names.append(name)
            out_avals.append(jax.core.ShapedArray(
                tuple(alloc.tensor_shape), mybir.dt.np(alloc.dtype)))
    all_in_names = list(in_names) + list(out_names)
    if partition_name is not None:
        all_in_names.append(partition_name)
    all_in_names = tuple(all_in_names)

    def _body(*args):
        operands = list(args)
        if partition_name is not None:
            operands.append(partition_id_tensor())
        outs = _bass_exec_p.bind(
            *operands, out_avals=tuple(out_avals), in_names=all_in_names,
            out_names=tuple(out_names), lowering_input_output_aliases=(),
            sim_require_finite=True, sim_require_nnan=True, nc=nc)
        return tuple(outs)

    devices = jax.devices()[:NCORES]
    mesh = Mesh(np.asarray(devices), ("core",))
    nio = len(in_names) + len(out_names)
    sharded = jax.jit(
        shard_map(_body, mesh=mesh,
                  in_specs=(PartitionSpec("core"),) * nio,
                  out_specs=(PartitionSpec("core"),) * len(out_names),
                  check_rep=False),
        keep_unused=True)
    sh = NamedSharding(mesh, PartitionSpec("core"))
    return sharded, in_names, out_avals, mesh, sh


def _get_rt(reps=1):
    key = ("rt", reps)
    if key not in _RT:
        nc = _build_nc(reps)
        sharded, in_names, out_avals, mesh, sh = _make_runner(nc)
        _RT[key] = dict(nc=nc, sharded=sharded, in_names=in_names,
                        out_avals=out_avals, mesh=mesh, sh=sh)
    return _RT[key]


def _stage_weights(rt, inputs):
    import jax
    fp = _fingerprint(inputs)
    if _RT.get("wfp") == fp:
        return
    maps = _prep_weight_maps(inputs)
    dw = {k: jax.device_put(v, rt["sh"]) for k, v in maps.items()}
    jax.block_until_ready(list(dw.values()))
    _RT["dweights"] = dw
    _RT["wfp"] = fp


def _get_zeros(rt):
    import jax
    if "dzeros" not in _RT:
        zs = [jax.device_put(
            np.zeros((NCORES * a.shape[0],) + a.shape[1:], a.dtype), rt["sh"])
            for a in rt["out_avals"]]
        jax.block_until_ready(zs)
        _RT["dzeros"] = zs
    return _RT["dzeros"]


def kernel(**inputs) -> np.ndarray:
    import jax
    rt = _get_rt(1)
    _stage_weights(rt, inputs)
    zeros = _get_zeros(rt)
    dw = _RT["dweights"]

    x = np.asarray(inputs["x"], np.float32).reshape(NT, D)
    dx = jax.device_put(x, rt["sh"])

    args = []
    for n in rt["in_names"]:
        args.append(dx if n == "xin" else dw[n])
    outs = rt["sharded"](*args, *zeros)
    o = np.asarray(outs[0])            # [NT, D] token-major
    return np.ascontiguousarray(o).reshape(B, S, D)
